# revision 48
# baseline (speedup 1.0000x reference)
"""GRU cell kernel for Trainium2, data-parallel over 8 NeuronCores.

Reference computation (B=4096, I=H=2048, C=I+H=4096):
    combined   = [x, h]                                   [B, C]
    to_update  = sigmoid(combined @ W_update.T + b_u)     [B, H]
    to_select  = sigmoid(combined @ W_select.T + b_s)     [B, H]
    updated    = h * to_update
    new_comb   = [x, updated]
    predictions= tanh(new_comb @ W_predict.T + b_p)
    h_new      = h * (1 - to_select) + predictions * to_select

Sharding: batch split 8 ways (512 rows/core), weights replicated.
On-chip layout is [feature, batch] (transposed), so each weight tile
[128c, 128h] is the stationary matmul operand and activation tiles
[128c, 512b] are the moving operand -- no on-chip transposes anywhere.
Matmuls run in bf16 (inputs host-cast) with fp32 PSUM accumulation;
the final blend uses fp32 h.

Latency structure (per core):
  * 1536 matmuls x ~213 ns is the PE floor (~329 us warm); everything
    else is arranged to keep PE busy from ~1 us onward.  Throwaway
    warm-up matmuls fill the DMA-priming window so the HAM/p-state ramp
    completes before the first real matmul (worth ~2.8 us).
  * DMA descriptor processing costs ~0.6 us per DMA regardless of size,
    so x / h / biases are uploaded as a few 0.5-1 MB chunk DMAs into
    flat [128, n*512] SBUF tiles; per-c-tile views feed the matmuls
    (subtile deps let each matmul wait only on its own chunk).
  * h is uploaded twice (bf16 for matmuls, fp32 for the blend) so no
    on-chip cast sits on the critical path; the fp32 copy is issued
    after the update phase (only the blend needs it).
  * The first PRE=8 update-gate gemms run their x-half contraction
    first, emitted as 2-matmul units in upload-readiness order across
    all 8 PSUM banks in step with the interleaved weight/x upload
    stream, so the PE starts after ~0.7 MB of DMA and has ~27 us of
    x-part work buffered against upload jitter before it needs h.
  * h*(1-sel) is precomputed into the fp32 h tile during the select
    phase (DVE slack), so the predict-phase tail per tile is
    tanh + 2 DVE ops + store; the last output tile is split into a wide
    and a narrow PSUM group so only a short final chain stays exposed.
"""

from contextlib import ExitStack

import numpy as np
import ml_dtypes

import concourse.tile as tile
import concourse.mybir as mybir
from concourse import bacc
from concourse.bass_utils import run_bass_kernel_spmd

BF16 = mybir.dt.bfloat16
F32 = mybir.dt.float32
NPBF16 = ml_dtypes.bfloat16

B, I, H = 4096, 2048, 2048
C = I + H
NCORES = 8
BS = B // NCORES            # 512 batch rows per core
P = 128                     # SBUF partitions
HT = H // P                 # 16 output-row tiles
IT = I // P                 # 16 x feature tiles
CT = C // P                 # 32 contraction tiles
HALF = C // 2
PRE = 8                     # update-gate gemms with split x/h contraction
NXC = 6                     # bf16 x upload chunks (2 c-tiles each)
NCH = 4                     # h upload chunks (4 c-tiles, 0.5/1 MB each)
NF8 = 4                     # x c-tiles 0..3 run as fp8 DoubleRow (2 MMs)
NH8 = 2                     # h c-tiles 16..17 fp8 on update/select gemms only
WSCALE = 64.0               # weights packed x64 (exact bf16 exponent shift);
                            # undone by scale=1/64 in the activations
NXB = IT - NF8              # bf16 x c-tiles (4..15)
CHT = IT // NCH             # c-tiles per h chunk
ACT_F = mybir.ActivationFunctionType

_PROGRAM = None


def _build_program():
    nc = bacc.Bacc("TRN2")

    xTd = nc.dram_tensor("xTd", [P, NXB * BS], BF16, kind="ExternalInput")
    F8 = mybir.dt.float8e4
    xf8d = nc.dram_tensor("xf8d", [P, NF8 // 2, 2, BS], F8, kind="ExternalInput")
    Wuf8 = nc.dram_tensor("Wuf8", [HT, P, NF8 // 2, 2, P], F8, kind="ExternalInput")
    Wsf8 = nc.dram_tensor("Wsf8", [HT, P, NF8 // 2, 2, P], F8, kind="ExternalInput")
    Wpf8 = nc.dram_tensor("Wpf8", [HT, P, NF8 // 2, 2, P], F8, kind="ExternalInput")
    hf8d = nc.dram_tensor("hf8d", [P, NH8 // 2, 2, BS], F8, kind="ExternalInput")
    Wuh8 = nc.dram_tensor("Wuh8", [HT, P, NH8 // 2, 2, P], F8, kind="ExternalInput")
    Wsh8 = nc.dram_tensor("Wsh8", [HT, P, NH8 // 2, 2, P], F8, kind="ExternalInput")
    hTb = nc.dram_tensor("hTb", [P, H // P * BS], BF16, kind="ExternalInput")
    hT32 = nc.dram_tensor("hT32", [P, H // P * BS], F32, kind="ExternalInput")
    Wu = nc.dram_tensor("Wu", [HT, P, C], BF16, kind="ExternalInput")
    Ws = nc.dram_tensor("Ws", [HT, P, C], BF16, kind="ExternalInput")
    Wp = nc.dram_tensor("Wp", [HT, P, C], BF16, kind="ExternalInput")
    bias = nc.dram_tensor("bias", [P, 3 * HT], F32, kind="ExternalInput")
    WF8 = {id(Wu): Wuf8, id(Ws): Wsf8, id(Wp): Wpf8}
    WH8 = {id(Wu): Wuh8, id(Ws): Wsh8}
    out = nc.dram_tensor("out", [HT, P, BS], F32, kind="ExternalOutput")

    CW = CHT * BS  # flat columns per upload chunk

    with tile.TileContext(nc) as tc, ExitStack() as ctx:
        singles = ctx.enter_context(tc.tile_pool(name="singles", bufs=1))
        wpool = ctx.enter_context(tc.tile_pool(name="wpool", bufs=6))
        # 6 full-bank accumulation tiles + 2 sub-bank ones (last tile,
        # bank-padded) = 8 PSUM banks exactly
        pspool = ctx.enter_context(tc.tile_pool(name="ps", bufs=6, space="PSUM"))
        work = ctx.enter_context(tc.tile_pool(name="work", bufs=4))

        bias_sb = singles.tile([P, 3 * HT], F32, name="bias_sb")
        bu_sb = bias_sb[:, 0:HT]
        bs_sb = bias_sb[:, HT:2 * HT]
        bp_sb = bias_sb[:, 2 * HT:3 * HT]

        # PE warm-up: throwaway matmuls on a zeroed scratch tile while the
        # first real operands upload (~4 us of dead PE time).  They keep the
        # HAM/p-state activity window running so the first real matmuls issue
        # at full clock instead of paying the ~1.2 GHz cold ramp.  Sized to
        # finish just before the first real operands land even when starting
        # cold.  The PSUM bank is a "pshalf" slot reused (and re-cleared by
        # start=True) much later by the last predict tile.
        NWARM = 32
        scratch = singles.tile([P, P], BF16, name="scratch")
        nc.vector.memset(scratch[:], 0.0)
        psd = pspool.tile([P, P], F32, tag="pshalf", name="psd", bufs=2)
        for k in range(NWARM):
            nc.tensor.matmul(psd, scratch[:], scratch[:],
                             start=(k == 0), stop=(k == NWARM - 1))

        xsb = singles.tile([P, NXB * BS], BF16, name="xsb")
        xf8 = singles.tile([P, NF8 // 2, 2, BS], F8, name="xf8")
        hf8 = singles.tile([P, NH8 // 2, 2, BS], F8, name="hf8")
        hbsb = singles.tile([P, H // P * BS], BF16, name="hbsb")
        h32sb = singles.tile([P, H // P * BS], F32, name="h32sb")

        # Interleave 0.25 MB x-chunk and half-weight-block uploads 1:1 so the
        # first matmuls start after ~0.7 MB of DMA and the PE stays fed chunk
        # by chunk (subtile deps: each matmul waits only on the piece that
        # covers its columns).  `rank` records upload order for the greedy
        # matmul emission below.
        XW = 2 * BS                      # flat columns per x chunk
        # "wx" ring must hold all PRE prologue blocks live at once, plus one
        # slot of prefetch slack for the i >= PRE gemms
        WXW = NXB * P                    # bf16 x-part weight columns
        wxs = [
            wpool.tile([P, WXW], BF16, tag="wx", name="wx", bufs=PRE + 1)
            for _ in range(PRE)
        ]
        # Hand-tuned order (PRE=8, NXC=8): x1 right after x0 so a second unit
        # is ready when the first is burned; half-1 blocks deferred (useless
        # until x4); groups 4-7 and the bf16 h chunks trail interleaved.
        stream = [("w", 0, 0), ("x", 0), ("x", 1), ("w", 1, 0), ("x", 2),
                  ("w", 2, 0), ("x", 3), ("w", 3, 0), ("x", 4), ("w", 0, 1),
                  ("x", 5), ("w", 1, 1), ("w", 2, 1), ("w", 3, 1),
                  ("w", 4, 0), ("f8",), ("h", 0), ("w", 4, 1), ("w", 5, 0),
                  ("h", 1), ("w", 5, 1), ("w", 6, 0), ("h", 2), ("w", 6, 1),
                  ("w", 7, 0), ("h", 3), ("w", 7, 1)]
        assert sorted(e[1] for e in stream if e[0] == "x") == list(range(NXC))
        assert sum(1 for e in stream if e[0] == "f8") == 1
        assert sorted(e[1] for e in stream if e[0] == "h") == list(range(NCH))
        assert sorted((e[1], e[2]) for e in stream if e[0] == "w") == [
            (i, half) for i in range(PRE) for half in range(2)
        ]
        x_rank = {}
        wx_rank = {}
        for rank, e in enumerate(stream):
            if e[0] == "x":
                c = e[1]
                nc.sync.dma_start(xsb[:, c * XW:(c + 1) * XW],
                                  xTd[:, c * XW:(c + 1) * XW])
                x_rank[c] = rank
            elif e[0] == "h":
                c = e[1]
                nc.sync.dma_start(hbsb[:, c * CW:(c + 1) * CW],
                                  hTb[:, c * CW:(c + 1) * CW])
            elif e[0] == "f8":
                nc.sync.dma_start(xf8[:], xf8d[:])
                nc.sync.dma_start(hf8[:], hf8d[:])
            else:
                _, i, half = e
                cols = slice(half * WXW // 2, (half + 1) * WXW // 2)
                nc.sync.dma_start(wxs[i][:, cols],
                                  Wu[i, :, NF8 * P + cols.start:
                                     NF8 * P + cols.stop])
                wx_rank[(i, half)] = rank
        # biases are first needed by the sigmoid at ~30 us; keep their DMA
        # out of the critical prologue stream.
        nc.sync.dma_start(bias_sb[:], bias[:])

        # combined.T views: c-tiles 0..3 live in xf8 (DoubleRow pairs); 4..15
        # in xsb; 16..31 in hbsb
        comb = [None] * NF8
        comb += [xsb[:, n * BS:(n + 1) * BS] for n in range(NXB)]
        comb += [hbsb[:, i * BS:(i + 1) * BS] for i in range(HT)]
        h32 = [h32sb[:, i * BS:(i + 1) * BS] for i in range(HT)]

        upd = [
            singles.tile([P, BS], BF16, name=f"upd{i}", tag=f"upd{i}")
            for i in range(HT)
        ]
        selb = [
            singles.tile([P, BS], BF16, name=f"selb{i}", tag=f"selb{i}")
            for i in range(HT)
        ]

        def load_w(W, i, h8=False):
            wx = wpool.tile([P, WXW], BF16, tag="wx", name="wx", bufs=PRE + 1)
            nc.sync.dma_start(wx[:], W[i, :, NF8 * P:HALF])
            wf8 = wpool.tile([P, NF8 // 2, 2, P], F8, tag="wf8", name="wf8")
            nc.sync.dma_start(wf8[:], WF8[id(W)][i])
            if h8:
                return (wx, wf8) + load_wh8(W, i)
            wh = wpool.tile([P, HALF], BF16, tag="wh", name="wh")
            nc.sync.dma_start(wh[:], W[i, :, HALF:C])
            return wx, wf8, wh

        def load_wh8(W, i):
            wh8 = wpool.tile([P, NH8 // 2, 2, P], F8, tag="wh8", name="wh8")
            nc.sync.dma_start(wh8[:], WH8[id(W)][i])
            whr = wpool.tile([P, HALF - NH8 * P], BF16, tag="wh", name="whr")
            nc.sync.dma_start(whr[:], W[i, :, HALF + NH8 * P:C])
            return wh8, whr

        def mm_h8(ps, wh8, whr, stop, cols=None):
            """update/select h-part: 1 DoubleRow pair (c16..17, hf8) + bf16
            matmuls for c-tiles 18..31."""
            for k in range(NH8 // 2):
                r = hf8[:, k]
                nc.tensor.matmul(
                    ps, wh8[:, k], r if cols is None else r[:, :, cols],
                    start=False, stop=False,
                    perf_mode=mybir.MatmulPerfMode.DoubleRow,
                )
            for n in range(IT + NH8, CT):
                w_ap = whr[:, (n - IT - NH8) * P:(n - IT - NH8 + 1) * P]
                r = comb[n]
                nc.tensor.matmul(
                    ps, w_ap, r if cols is None else r[:, cols],
                    start=False, stop=(stop and n == CT - 1),
                )

        def load_wf8(W, i):
            wf8 = wpool.tile([P, NF8 // 2, 2, P], F8, tag="wf8", name="wf8")
            nc.sync.dma_start(wf8[:], WF8[id(W)][i])
            return wf8

        def mm_f8(ps, wf8, start, cols=None):
            """c-tiles 0..3 as fp8 DoubleRow pairs (x operand = xf8)."""
            for k in range(NF8 // 2):
                r = xf8[:, k]
                nc.tensor.matmul(
                    ps, wf8[:, k], r if cols is None else r[:, :, cols],
                    start=(start and k == 0), stop=False,
                    perf_mode=mybir.MatmulPerfMode.DoubleRow,
                )

        def mm_half(ps, w, rhs_tiles, n0, n1, start, stop, cols=None):
            for n in range(n0, n1):
                w_ap = w[:, (n - n0) * P:(n - n0 + 1) * P]
                r = rhs_tiles[n]
                nc.tensor.matmul(
                    ps,
                    w_ap,
                    r if cols is None else r[:, cols],
                    start=(start and n == n0),
                    stop=(stop and n == n1 - 1),
                )

        # ---- update gate: upd[i] = h * sigmoid(z_u) ----
        # First PRE gemms: x-half contraction, emitted as 2-matmul units in
        # upload-readiness order across the PRE PSUM banks.
        # 6 full-bank "ps" slots + the 2 bank-padded "pshalf" slots (reused by
        # the last predict tile much later) give 8 concurrent accumulators.
        psA = []
        for i in range(PRE):
            if i < 6:
                ps = pspool.tile([P, BS], F32, tag="ps", name="ps")
            else:
                ps = pspool.tile([P, BS], F32, tag="pshalf", name="ps", bufs=2)
            psA.append(ps)
        units = sorted(
            ((max(x_rank[c], wx_rank[(i, c // (NXC // 2))]), i, c)
             for i in range(PRE) for c in range(NXC)),
            key=lambda u: (u[0], u[2], u[1]),
        )
        started = set()
        for _, i, c in units:
            for n in (NF8 + 2 * c, NF8 + 2 * c + 1):
                nc.tensor.matmul(
                    psA[i], wxs[i][:, (n - NF8) * P:(n - NF8 + 1) * P], comb[n],
                    start=(i not in started), stop=False,
                )
                started.add(i)

        def finish_update(i, ps):
            u = work.tile([P, BS], BF16, tag="u", name="u")
            nc.scalar.activation(u[:], ps[:], ACT_F.Sigmoid,
                                 bias=bu_sb[:, i:i + 1], scale=1.0 / WSCALE)
            nc.vector.tensor_mul(upd[i][:], comb[IT + i], u[:])

        for i in range(PRE):
            wf8 = load_wf8(Wu, i)
            mm_f8(psA[i], wf8, start=False)
            wh8, whr = load_wh8(Wu, i)
            mm_h8(psA[i], wh8, whr, stop=True)
            finish_update(i, psA[i])

        for i in range(PRE, HT):
            wx, wf8, _wh8, whr = load_w(Wu, i, h8=True)
            ps = pspool.tile([P, BS], F32, tag="ps", name="ps")
            mm_f8(ps, wf8, start=True)
            mm_half(ps, wx, comb, NF8, IT, start=False, stop=False)
            mm_h8(ps, _wh8, whr, stop=True)
            finish_update(i, ps)

        # fp32 h: only needed from the select phase on (blend terms), so its
        # upload is issued after the update-phase weight loads.
        for c in range(NCH):
            nc.sync.dma_start(h32sb[:, c * CW:(c + 1) * CW],
                              hT32[:, c * CW:(c + 1) * CW])

        # ---- select gate ----
        # sel kept bf16 (it only multiplies |tanh| <= 1 in the blend); the
        # numerically sensitive term h*(1-sel) is computed here in fp32 and
        # overwrites h32[i] in place (h itself is not needed afterwards).
        for i in range(HT):
            wx, wf8, _wh8, whr = load_w(Ws, i, h8=True)
            ps = pspool.tile([P, BS], F32, tag="ps", name="ps")
            mm_f8(ps, wf8, start=True)
            mm_half(ps, wx, comb, NF8, IT, start=False, stop=False)
            mm_h8(ps, _wh8, whr, stop=True)
            s32 = work.tile([P, BS], F32, tag="s32", name="s32")
            nc.scalar.activation(s32[:], ps[:], ACT_F.Sigmoid,
                                 bias=bs_sb[:, i:i + 1], scale=1.0 / WSCALE)
            nc.vector.tensor_copy(selb[i][:], s32[:])
            nc.vector.tensor_mul(s32[:], h32[i], s32[:])
            nc.vector.tensor_sub(h32[i], h32[i], s32[:])

        hs = h32  # h32[i] now holds h * (1 - sel)

        # ---- predictions + blend: h_new = hs + sel * tanh(z_p) ----
        newcomb = comb[:IT] + [upd[i][:] for i in range(HT)]

        def blend(i, ps_ap, cols, otag):
            n = cols.stop - cols.start
            p_t = work.tile([P, n], F32, tag=f"p{otag}", name="p_t")
            nc.scalar.activation(p_t[:], ps_ap, ACT_F.Tanh,
                                 bias=bp_sb[:, i:i + 1], scale=1.0 / WSCALE)
            o = work.tile([P, n], F32, tag=f"o{otag}", name="o")
            nc.vector.tensor_mul(o[:], p_t[:], selb[i][:, cols])
            nc.vector.tensor_add(o[:], o[:], hs[i][:, cols])
            nc.sync.dma_start(out[i, :, cols], o[:])

        for i in range(HT - 1):
            wx, wf8, wh = load_w(Wp, i)
            ps = pspool.tile([P, BS], F32, tag="ps", name="ps")
            mm_f8(ps, wf8, start=True)
            mm_half(ps, wx, newcomb, NF8, IT, start=False, stop=False)
            mm_half(ps, wh, newcomb, IT, CT, start=False, stop=True)
            blend(i, ps[:], slice(0, BS), "f")

        # Last tile: two unequal batch-column accumulation groups.  The wide
        # group's tanh+blend+store overlaps the narrow group's matmuls, and
        # the narrow group leaves only a short exposed final chain.
        i = HT - 1
        wx, wf8, wh = load_w(Wp, i)
        for cols in (slice(0, 3 * BS // 4), slice(3 * BS // 4, BS)):
            ps = pspool.tile([P, cols.stop - cols.start], F32, tag="pshalf",
                             name="pshalf", bufs=2)
            mm_f8(ps, wf8, start=True, cols=cols)
            mm_half(ps, wx, newcomb, NF8, IT, start=False, stop=False, cols=cols)
            mm_half(ps, wh, newcomb, IT, CT, start=False, stop=True, cols=cols)
            blend(i, ps[:], cols, "h")

    nc.finalize()
    return nc


def _get_program():
    global _PROGRAM
    if _PROGRAM is None:
        _PROGRAM = _build_program()
    return _PROGRAM


def _pack_weight(w):
    """[H, C] fp32 -> [HT, P, C] bf16 with [i, p, n*128+m] = W[i*128+m, n*128+p].

    Slice [i] is then an SBUF block whose column window n*128:(n+1)*128 is the
    stationary operand (lhsT = W.T tile) for contraction tile n.
    """
    w64 = np.asarray(w, dtype=np.float32) * WSCALE
    packed = np.ascontiguousarray(
        w64.reshape(HT, P, CT, P).transpose(0, 3, 2, 1).reshape(HT, P, C)
    )
    wf8 = np.ascontiguousarray(
        packed[:, :, 0:NF8 * P].astype(ml_dtypes.float8_e4m3)
        .reshape(HT, P, NF8 // 2, 2, P)
    )
    wh8 = np.ascontiguousarray(
        packed[:, :, HALF:HALF + NH8 * P].astype(ml_dtypes.float8_e4m3)
        .reshape(HT, P, NH8 // 2, 2, P)
    )
    return packed.astype(NPBF16), wf8, wh8


def _pack_act(a, np_dtype):
    """[BS, F] -> flat [P, F//P * BS] with [p, n*BS+b] = a[b, n*128+p]."""
    ft = a.shape[1] // P
    return np.ascontiguousarray(
        np.asarray(a, dtype=np_dtype).reshape(BS, ft, P).transpose(2, 1, 0)
        .reshape(P, ft * BS)
    )


def _prep_inputs(x, h, W_update, b_update, W_select, b_select, W_predict, b_predict):
    x = np.asarray(x, dtype=np.float32)
    h = np.asarray(h, dtype=np.float32)

    Wu, Wuf8, Wuh8 = _pack_weight(W_update)
    Ws, Wsf8, Wsh8 = _pack_weight(W_select)
    Wp, Wpf8, _ = _pack_weight(W_predict)
    bias = np.ascontiguousarray(
        np.concatenate(
            [
                np.asarray(b, dtype=np.float32).reshape(HT, P).T
                for b in (b_update, b_select, b_predict)
            ],
            axis=1,
        )
    )

    in_maps = []
    for c in range(NCORES):
        rows = slice(c * BS, (c + 1) * BS)
        xpack = _pack_act(x[rows], np.float32)
        in_maps.append(
            {
                "xTd": np.ascontiguousarray(
                    xpack[:, NF8 * BS:].astype(NPBF16)),
                "xf8d": np.ascontiguousarray(
                    xpack[:, :NF8 * BS].astype(ml_dtypes.float8_e4m3)
                    .reshape(P, NF8 // 2, 2, BS)),
                "hTb": _pack_act(h[rows], NPBF16),
                "hT32": _pack_act(h[rows], np.float32),
                "hf8d": np.ascontiguousarray(
                    _pack_act(h[rows], np.float32)[:, :NH8 * BS]
                    .astype(ml_dtypes.float8_e4m3)
                    .reshape(P, NH8 // 2, 2, BS)),
                "Wuh8": Wuh8, "Wsh8": Wsh8,
                "Wu": Wu, "Wuf8": Wuf8,
                "Ws": Ws, "Wsf8": Wsf8,
                "Wp": Wp, "Wpf8": Wpf8,
                "bias": bias,
            }
        )
    return in_maps


def kernel(x, h, W_update, b_update, W_select, b_select, W_predict, b_predict,
           _trace=False):
    nc = _get_program()
    in_maps = _prep_inputs(
        x, h, W_update, b_update, W_select, b_select, W_predict, b_predict
    )
    res = run_bass_kernel_spmd(
        nc, in_maps, core_ids=list(range(NCORES)), trace=_trace
    )
    h_new = np.empty((B, H), dtype=np.float32)
    for c in range(NCORES):
        rows = slice(c * BS, (c + 1) * BS)
        h_new[rows] = res.results[c]["out"].reshape(H, BS).T
    if _trace:
        return h_new, res
    return h_new
